# revision 19
# baseline (speedup 1.0000x reference)
"""BertAttention (with additive KV injection) Trainium2 kernel.

Problem: nn_BertAttention_12781822673413
  B=4, S=2048, DM=768, H=12 heads, HD=64, NSYN=4 (additive k/v on first 4 heads)
  out = LayerNorm(attn_out @ Wo.T + bo + x) * ln_g + ln_b

Sharding: 8 cores = (batch b, query-half) pairs.  Each core computes q for its
1024-token half, k/v for the full 2048 sequence of its batch (k/v projection is
duplicated across the 2 cores of a batch - this avoids any collective), runs
12 heads of attention for its query half, output projection, residual + LN.
No collectives; outputs are disjoint slices of the full output.

Precision plan: projections and the probs@V matmul run in fp8(e4m3) with
DoubleRow perf mode (2 K-rows per PE cell -> half the matmul cycles); scores
(K=64) stay bf16; all psum accumulation is fp32; softmax, normalize, residual
and LayerNorm are fp32.  Weights are host-scaled by 16 to sit in fp8's normal
range (std 0.02 -> 0.32); the scale is tracked exactly through the pipeline
(q,k carry 16x, scores 256x -> exp scale /256; v carries 16x, the v_aug ones
column is 1/16 so the denominator carries 1/16 -> normalized ctx carries 256x;
the out projection's 16x weights make psum 4096x, removed by a *2^-12 in the
residual add).  All scale factors are powers of two, hence exact.

The softmax denominator falls out of the PV matmul itself: v_aug has a
per-head 65th column holding 1/16, so ctx-psum row 64 is the (scaled) sum of
exp.  The normalize chain (DVE reciprocal -> DMA broadcast via a DRAM bounce
-> one DVE multiply straight out of psum) never touches the PE and runs one
head behind, so the PE stream is pure back-to-back matmuls and the HAM clock
gate stays at 2.4GHz (any >3.4us PE idle gap throttles it to 1.2GHz).

The zero-valued biases (bq,bk,bv) get dedicated instructions only when nonzero
(decided at trace time from the actual input values); bo is folded into the
residual input on the host; ln_g/ln_b are applied on the host when nontrivial.
"""

import os
import sys

for _p in ("/opt/trn_rl_repo", "/root/.axon_site/_ro/trn_rl_repo"):
    if os.path.isdir(_p) and _p not in sys.path:
        sys.path.insert(0, _p)

from contextlib import ExitStack

import ml_dtypes
import numpy as np

import concourse.bass as bass
import concourse.tile as tile
from concourse import bacc, mybir
from concourse.bass_utils import run_bass_kernel_spmd

BF16 = ml_dtypes.bfloat16
FP8 = ml_dtypes.float8_e4m3

B, S, DM, H, NSYN = 4, 2048, 768, 12, 4
HD = DM // H            # 64
SH = S // 2             # 1024 queries per core
P = 128
NT = S // P             # 16 key tiles
NJ = DM // P            # 6 model-dim tiles
NI2 = DM // 256         # 3 DoubleRow contraction tiles (256 each)
NSH = SH // P           # 8 query tiles
SCALE = float(DM / H) ** -0.5   # 0.125
EPS = 1e-12
N_CORES = 8
WS = 16.0               # host-side weight scale (fp8 range)
VA2W = 784              # padded v_aug row width (12*65 -> %16 for DoubleRow AP)
HEAD_ORDER = list(range(2, H)) + [0, 1]   # heads 0,1 last -> their ctx tile
IT2_ORDER = [1, 2, 0]                     # is contracted last in the out proj

f32 = mybir.dt.float32
bf16 = mybir.dt.bfloat16
fp8 = mybir.dt.float8e4

AF = mybir.ActivationFunctionType
ALU = mybir.AluOpType
DR = mybir.MatmulPerfMode.DoubleRow


def _build_program(bq_nz: bool, bk_nz: bool, bv_nz: bool, mask_nz: bool = False):
    nc = bacc.Bacc(
        "TRN2",
        target_bir_lowering=False,
        debug=False,
        enable_asserts=False,
        num_devices=N_CORES,
    )

    xT = nc.dram_tensor("xT", [P, NI2, 2, S], fp8, kind="ExternalInput").ap()
    xr = nc.dram_tensor("xr", [SH, DM], f32, kind="ExternalInput").ap()
    wq = nc.dram_tensor("wqT", [P, NI2, 2, DM], fp8, kind="ExternalInput").ap()
    wk = nc.dram_tensor("wkT", [P, NI2, 2, DM], fp8, kind="ExternalInput").ap()
    wv = nc.dram_tensor("wvT", [P, NI2, 2, DM], fp8, kind="ExternalInput").ap()
    wo = nc.dram_tensor("woT", [P, NI2, 2, DM], fp8, kind="ExternalInput").ap()
    addikT = nc.dram_tensor("addikT", [NSYN * HD, S], bf16, kind="ExternalInput").ap()
    addiv = nc.dram_tensor("addiv", [S, NSYN * HD], bf16, kind="ExternalInput").ap()
    maskd = nc.dram_tensor("mask", [S], f32, kind="ExternalInput").ap()
    bqd = nc.dram_tensor("bq", [DM], f32, kind="ExternalInput").ap()
    bkd = nc.dram_tensor("bk", [DM], f32, kind="ExternalInput").ap()
    bvd = nc.dram_tensor("bv", [DM], f32, kind="ExternalInput").ap()
    out = nc.dram_tensor("out", [SH, DM], f32, kind="ExternalOutput").ap()

    with tile.TileContext(nc) as tc, ExitStack() as ctx:
        const = ctx.enter_context(tc.tile_pool(name="const", bufs=1))

        xT_sb = const.tile([P, NI2, 2, S], fp8, name="xT_sb")
        wq_sb = const.tile([P, NI2, 2, DM], fp8, name="wq_sb")
        wk_sb = const.tile([P, NI2, 2, DM], fp8, name="wk_sb")
        wv_sb = const.tile([P, NI2, 2, DM], fp8, name="wv_sb")
        wo_sb = const.tile([P, NI2, 2, DM], fp8, name="wo_sb")
        qT_sb = const.tile([P, NJ, SH], bf16, name="qT_sb")
        kT_sb = const.tile([P, NJ, S], bf16, name="kT_sb")
        vaug_sb = const.tile([P, NT // 2, 2, VA2W], fp8, name="vaug_sb")
        ctxT_sb = const.tile([P, NI2, 2, SH], fp8, name="ctxT_sb")
        mask_sb = const.tile([P, NT], f32, name="mask_sb")
        eps_sb = const.tile([P, 1], f32, name="eps_sb")

        # DMA order: q/k weights and x first (they gate the first matmuls).
        for it2 in range(NI2):
            for pl in range(2):
                nc.sync.dma_start(wq_sb[:, it2, pl, :], wq[:, it2, pl, :])
                nc.sync.dma_start(xT_sb[:, it2, pl, :], xT[:, it2, pl, :])
                nc.sync.dma_start(wk_sb[:, it2, pl, :], wk[:, it2, pl, :])
        for it2 in range(NI2):
            for pl in range(2):
                nc.sync.dma_start(wv_sb[:, it2, pl, :], wv[:, it2, pl, :])
        for it2 in range(NI2):
            for pl in range(2):
                nc.sync.dma_start(wo_sb[:, it2, pl, :], wo[:, it2, pl, :])
        nc.sync.dma_start(mask_sb[:], maskd.rearrange("(t p) -> p t", p=P))
        nc.vector.memset(eps_sb[:], EPS)
        # ones columns of v_aug hold 1/16 (exact in fp8); the projection
        # writes below only cover offsets 0..63 of each 65-wide head block.
        nc.gpsimd.memset(vaug_sb[:], 1.0 / WS)

        bias_tiles = {}
        for nz, nm, dram in ((bq_nz, "bq", bqd), (bk_nz, "bk", bkd), (bv_nz, "bv", bvd)):
            if nz:
                t = const.tile([P, NJ], f32, name=f"{nm}_sb")
                nc.sync.dma_start(t[:], dram.rearrange("(t p) -> p t", p=P))
                bias_tiles[nm] = t

        ps = ctx.enter_context(tc.tile_pool(name="ps", bufs=2, space="PSUM"))
        psc = ctx.enter_context(tc.tile_pool(name="psc", bufs=2, space="PSUM"))
        ppool = ctx.enter_context(tc.tile_pool(name="ppool", bufs=4))
        akpool = ctx.enter_context(tc.tile_pool(name="akpool", bufs=2))
        avpool = ctx.enter_context(tc.tile_pool(name="avpool", bufs=2))
        rcpool = ctx.enter_context(tc.tile_pool(name="rcpool", bufs=2))
        bcpool = ctx.enter_context(tc.tile_pool(name="bcpool", bufs=2))
        drpool = ctx.enter_context(tc.tile_pool(name="drpool", bufs=2, space="DRAM"))
        xrpool = ctx.enter_context(tc.tile_pool(name="xrpool", bufs=2))
        hpool = ctx.enter_context(tc.tile_pool(name="hpool", bufs=2))
        opool = ctx.enter_context(tc.tile_pool(name="opool", bufs=2))
        stpool = ctx.enter_context(tc.tile_pool(name="stpool", bufs=3))

        def psum_tile(name):
            return ps.tile([P, 1024], f32, name=name, tag="ps")

        # ---- Phase 1a: qT[j, s] (16x scaled; own query half = xT cols 0:1024)
        for jt in [1, 2, 3, 4, 5, 0]:
            psq = psum_tile(f"psq{jt}")
            for i, it2 in enumerate(range(NI2)):
                lhs = wq_sb[:, it2, :, jt * P : (jt + 1) * P]
                for c0 in (0, 512):
                    nc.tensor.matmul(
                        psq[:, c0 : c0 + 512],
                        lhsT=lhs,
                        rhs=xT_sb[:, it2, :, c0 : c0 + 512],
                        start=(i == 0),
                        stop=(i == NI2 - 1),
                        perf_mode=DR,
                    )
            dest = qT_sb[:, jt, :]
            if bq_nz:
                nc.scalar.activation(
                    dest, psq[:], AF.Identity, bias=bias_tiles["bq"][:, jt : jt + 1]
                )
            else:
                nc.any.tensor_copy(out=dest, in_=psq[:])

        # ---- Phase 1b: kT[j, t] (16x; + additive key on heads 0..3) ----
        for jt in [1, 0, 2, 3, 4, 5]:
            for th in range(2):
                psk = psum_tile(f"psk{jt}_{th}")
                for i, it2 in enumerate(range(NI2)):
                    lhs = wk_sb[:, it2, :, jt * P : (jt + 1) * P]
                    for c0 in (0, 512):
                        nc.tensor.matmul(
                            psk[:, c0 : c0 + 512],
                            lhsT=lhs,
                            rhs=xT_sb[:, it2, :, th * 1024 + c0 : th * 1024 + c0 + 512],
                            start=(i == 0),
                            stop=(i == NI2 - 1),
                            perf_mode=DR,
                        )
                dest = kT_sb[:, jt, th * 1024 : (th + 1) * 1024]
                if jt < 2:  # heads 0..3 live on partition tiles 0 and 1
                    ak = akpool.tile([P, 1024], bf16, name="ak", tag="ak")
                    nc.sync.dma_start(
                        ak[:],
                        addikT[jt * P : (jt + 1) * P, th * 1024 : (th + 1) * 1024],
                    )
                    nc.vector.tensor_add(out=dest, in0=psk[:], in1=ak[:])
                    if bk_nz:
                        nc.vector.tensor_scalar_add(
                            dest, dest, bias_tiles["bk"][:, jt : jt + 1]
                        )
                else:
                    if bk_nz:
                        nc.scalar.activation(
                            dest, psk[:], AF.Identity,
                            bias=bias_tiles["bk"][:, jt : jt + 1],
                        )
                    else:
                        nc.any.tensor_copy(out=dest, in_=psk[:])

        # ---- Phase 1c: v[t, j] (16x) into v_aug (+ additive value) ----
        for tt in range(NT):
            psv = psum_tile(f"psv{tt}")
            for i, it2 in enumerate(range(NI2)):
                lhs = xT_sb[:, it2, :, tt * P : (tt + 1) * P]
                nc.tensor.matmul(
                    psv[:, 0:512], lhsT=lhs, rhs=wv_sb[:, it2, :, 0:512],
                    start=(i == 0), stop=(i == NI2 - 1), perf_mode=DR,
                )
                nc.tensor.matmul(
                    psv[:, 512:768], lhsT=lhs, rhs=wv_sb[:, it2, :, 512:768],
                    start=(i == 0), stop=(i == NI2 - 1), perf_mode=DR,
                )
            vrow = vaug_sb[:, tt // 2, tt % 2, : H * (HD + 1)].rearrange(
                "p (h e) -> p h e", e=HD + 1
            )
            av = avpool.tile([P, NSYN * HD], bf16, name="av", tag="av")
            nc.sync.dma_start(av[:], addiv[tt * P : (tt + 1) * P, :])
            nc.vector.tensor_add(
                out=vrow[:, 0:NSYN, 0:HD],
                in0=psv[:, 0 : NSYN * HD].rearrange("p (h e) -> p h e", e=HD),
                in1=av[:].rearrange("p (h e) -> p h e", e=HD),
            )
            nc.any.tensor_copy(
                out=vrow[:, NSYN:H, 0:HD],
                in_=psv[:, NSYN * HD : DM].rearrange("p (h e) -> p h e", e=HD),
            )

        # ---- Phase 2: attention per head ----
        ctx_tiles = {}

        def normalize(h):
            it2, pl, po = h // 4, (h % 4) // 2, (h % 2) * HD
            psctx, bc = ctx_tiles.pop(h)
            dest = ctxT_sb[po : po + HD, it2, pl, :]
            nc.vector.tensor_mul(out=dest, in0=psctx[0:HD, :], in1=bc[:])
            if bv_nz:
                nc.vector.tensor_scalar_add(
                    dest, dest, bias_tiles["bv"][po : po + HD, h // 2 : h // 2 + 1]
                )

        for hi, h in enumerate(HEAD_ORDER):
            jt = h // 2
            po = (h % 2) * HD
            kTh = kT_sb[po : po + HD, jt, :]
            qTh = qT_sb[po : po + HD, jt, :]
            psctx = psc.tile([HD + 1, 1024], f32, name=f"ctx{h}", tag="ctx")
            for tp in range(NT // 2):
                pt2 = ppool.tile([P, 2, 1024], fp8, name="pt", tag="pt")
                for lo in (0, 1):
                    tt = 2 * tp + lo
                    pss = psum_tile(f"pss{h}_{tt}")
                    for c0 in (0, 512):
                        nc.tensor.matmul(
                            pss[:, c0 : c0 + 512],
                            lhsT=kTh[:, tt * P : (tt + 1) * P],
                            rhs=qTh[:, c0 : c0 + 512],
                            start=True,
                            stop=True,
                        )
                    if mask_nz:
                        nc.scalar.activation(
                            pt2[:, lo, :], pss[:], AF.Exp,
                            bias=mask_sb[:, tt : tt + 1], scale=SCALE / (WS * WS),
                        )
                    else:
                        nc.scalar.activation(
                            pt2[:, lo, :], pss[:], AF.Exp, scale=SCALE / (WS * WS)
                        )
                for c0 in (0, 512):
                    nc.tensor.matmul(
                        psctx[:, c0 : c0 + 512],
                        lhsT=vaug_sb[:, tp, :, h * (HD + 1) : h * (HD + 1) + HD + 1],
                        rhs=pt2[:, :, c0 : c0 + 512],
                        start=(tp == 0),
                        stop=(tp == NT // 2 - 1),
                        perf_mode=DR,
                    )
            if hi > 0:
                normalize(HEAD_ORDER[hi - 1])
            rc = rcpool.tile([1, 1024], f32, name="rc", tag="rc")
            nc.vector.reciprocal(rc[:], psctx[HD : HD + 1, :])
            dr = drpool.tile([1, 1024], f32, name="dr", tag="dr")
            nc.sync.dma_start(out=dr[:], in_=rc[:])
            bc = bcpool.tile([HD, 1024], f32, name="bc", tag="bc")
            nc.sync.dma_start(out=bc[:], in_=dr.to_broadcast((HD, 1024)))
            ctx_tiles[h] = (psctx, bc)
        normalize(HEAD_ORDER[-1])

        # ---- Phase 3: out proj (psum carries 4096x) + residual + LayerNorm
        for sc in range(NSH):
            pso = psum_tile(f"pso{sc}")
            for i, it2 in enumerate(IT2_ORDER):
                lhs = ctxT_sb[:, it2, :, sc * P : (sc + 1) * P]
                nc.tensor.matmul(
                    pso[:, 0:512], lhsT=lhs, rhs=wo_sb[:, it2, :, 0:512],
                    start=(i == 0), stop=(i == NI2 - 1), perf_mode=DR,
                )
                nc.tensor.matmul(
                    pso[:, 512:768], lhsT=lhs, rhs=wo_sb[:, it2, :, 512:768],
                    start=(i == 0), stop=(i == NI2 - 1), perf_mode=DR,
                )
            xrt = xrpool.tile([P, DM], f32, name="xrt", tag="xr")
            nc.sync.dma_start(xrt[:], xr[sc * P : (sc + 1) * P, :])
            ht = hpool.tile([P, DM], f32, name="ht", tag="h")
            nc.vector.scalar_tensor_tensor(
                out=ht[:], in0=pso[:, 0:DM], scalar=1.0 / 4096.0, in1=xrt[:],
                op0=ALU.mult, op1=ALU.add,
            )
            stats = stpool.tile([P, 3, 6], f32, name="stats", tag="st")
            for g in range(3):
                nc.vector.bn_stats(stats[:, g, :], ht[:, g * 256 : (g + 1) * 256])
            mv = stpool.tile([P, 2], f32, name="mv", tag="mv")
            nc.vector.bn_aggr(mv[:], stats[:])
            sq = stpool.tile([P, 1], f32, name="sq", tag="sq")
            nc.scalar.activation(sq[:], mv[:, 1:2], AF.Sqrt, bias=eps_sb[:])
            rstd = stpool.tile([P, 1], f32, name="rstd", tag="rstd")
            nc.vector.reciprocal(rstd[:], sq[:])
            ot = opool.tile([P, DM], f32, name="ot", tag="ot")
            nc.vector.tensor_scalar(
                out=ot[:], in0=ht[:],
                scalar1=mv[:, 0:1], scalar2=rstd[:],
                op0=ALU.subtract, op1=ALU.mult,
            )
            nc.sync.dma_start(out[sc * P : (sc + 1) * P, :], ot[:])

    nc.compile()
    return nc


_PROGRAM_CACHE: dict = {}


def _get_program(bq_nz, bk_nz, bv_nz, mask_nz=False):
    key = (bq_nz, bk_nz, bv_nz, mask_nz)
    if key not in _PROGRAM_CACHE:
        _PROGRAM_CACHE[key] = _build_program(*key)
    return _PROGRAM_CACHE[key]


def _dr_layout(a):
    """[256*NI2, N] -> [128, NI2, 2, N] DoubleRow K-interleave."""
    n = a.shape[1]
    return np.ascontiguousarray(a.reshape(NI2, 2, P, n).transpose(2, 0, 1, 3))


def _prep_core_inputs(inputs, b, half):
    """Host-side shard prep for core (b, half). Keys are permuted so the core's
    own query half comes first; attention is permutation-invariant in t as long
    as k, v, mask and the additive tensors share the order."""
    x = np.asarray(inputs["hidden_states"][b], np.float32)          # [S, DM]
    if half == 0:
        t_order = slice(None)
        xh = x[:SH]
    else:
        t_order = np.r_[SH:S, 0:SH]
        xh = x[SH:]
    xp = x[t_order] if half else x                                  # [S, DM] permuted
    xT = _dr_layout(np.ascontiguousarray(xp.T)).astype(FP8)
    xr = xh + np.asarray(inputs["bo"], np.float32)[None, :]         # residual + bo
    ak = np.asarray(inputs["addi_key"][b], np.float32) * WS         # [NSYN, S, HD]
    ak = ak.transpose(0, 2, 1).reshape(NSYN * HD, S)
    av = np.asarray(inputs["addi_value"][b], np.float32) * WS
    av = av.transpose(1, 0, 2).reshape(S, NSYN * HD)
    mask = np.asarray(inputs["attention_mask"][b, 0, 0], np.float32)
    if half:
        ak = ak[:, t_order]
        av = av[t_order]
        mask = mask[t_order]
    return {
        "xT": xT,
        "xr": np.ascontiguousarray(xr, np.float32),
        "addikT": np.ascontiguousarray(ak).astype(BF16),
        "addiv": np.ascontiguousarray(av).astype(BF16),
        "mask": np.ascontiguousarray(mask, np.float32),
    }


def _prep_in_maps(inputs):
    def w_prep(w):
        return _dr_layout(
            np.ascontiguousarray(np.asarray(w, np.float32).T) * WS
        ).astype(FP8)

    shared = {
        "wqT": w_prep(inputs["Wq"]),
        "wkT": w_prep(inputs["Wk"]),
        "wvT": w_prep(inputs["Wv"]),
        "woT": w_prep(inputs["Wo"]),
        # biases enter after the 16x-scaled projections / 256x-scaled ctx
        "bq": np.asarray(inputs["bq"], np.float32) * WS,
        "bk": np.asarray(inputs["bk"], np.float32) * WS,
        "bv": np.asarray(inputs["bv"], np.float32) * (WS * WS),
    }
    in_maps = []
    for c in range(N_CORES):
        m = _prep_core_inputs(inputs, c // 2, c % 2)
        m.update(shared)
        in_maps.append(m)
    return in_maps


def _postprocess(inputs, results):
    out = np.empty((B, S, DM), np.float32)
    for c in range(N_CORES):
        b, half = c // 2, c % 2
        out[b, half * SH : (half + 1) * SH] = results[c]["out"]
    ln_g = np.asarray(inputs["ln_g"], np.float32)
    ln_b = np.asarray(inputs["ln_b"], np.float32)
    if np.any(ln_b) or not np.all(ln_g == 1.0):
        out = out * ln_g[None, None, :] + ln_b[None, None, :]
    return out


def run(inputs, trace=False, **kwargs):
    """Run on hardware; returns (full_output, BassKernelResults)."""
    nc = _get_program(
        bool(np.any(inputs["bq"])),
        bool(np.any(inputs["bk"])),
        bool(np.any(inputs["bv"])),
        bool(np.any(inputs["attention_mask"])),
    )
    in_maps = _prep_in_maps(inputs)
    res = run_bass_kernel_spmd(
        nc, in_maps, core_ids=list(range(N_CORES)), trace=trace, **kwargs
    )
    return _postprocess(inputs, res.results), res


def kernel(**inputs) -> np.ndarray:
    out, _ = run(inputs)
    return out


# revision 25
# speedup vs baseline: 1.0407x; 1.0407x over previous
"""BertAttention (with additive KV injection) Trainium2 kernel.

Problem: nn_BertAttention_12781822673413
  B=4, S=2048, DM=768, H=12 heads, HD=64, NSYN=4 (additive k/v on first 4 heads)
  out = LayerNorm(attn_out @ Wo.T + bo + x) * ln_g + ln_b

Sharding: 8 cores = (batch b, query-half) pairs.  Each core computes q for its
1024-token half, k/v for the full 2048 sequence of its batch (k/v projection is
duplicated across the 2 cores of a batch - this avoids any collective), runs
12 heads of attention for its query half, output projection, residual + LN.
No collectives; outputs are disjoint slices of the full output.

Precision plan: projections and the probs@V matmul run in fp8(e4m3) with
DoubleRow perf mode (2 K-rows per PE cell -> half the matmul cycles); scores
(K=64) stay bf16; all psum accumulation is fp32; softmax, normalize, residual
and LayerNorm are fp32.  Weights are host-scaled by 16 to sit in fp8's normal
range (std 0.02 -> 0.32); the scale is tracked exactly through the pipeline
(q,k carry 16x, scores 256x -> exp scale /256; v carries 16x, the v_aug ones
column is 1/16 so the denominator carries 1/16 -> normalized ctx carries 256x;
the out projection's 16x weights make psum 4096x, removed by a *2^-12 in the
residual add).  All scale factors are powers of two, hence exact.

The softmax denominator falls out of the PV matmul itself: v_aug has a
per-head 65th column holding 1/16, so ctx-psum row 64 is the (scaled) sum of
exp.  The normalize chain (DVE reciprocal -> DMA broadcast via a DRAM bounce
-> one DVE multiply straight out of psum) never touches the PE and runs one
head behind, so the PE stream is pure back-to-back matmuls and the HAM clock
gate stays at 2.4GHz (any >3.4us PE idle gap throttles it to 1.2GHz).

The zero-valued biases (bq,bk,bv) get dedicated instructions only when nonzero
(decided at trace time from the actual input values); bo is folded into the
residual input on the host; ln_g/ln_b are applied on the host when nontrivial.
"""

import os
import sys

for _p in ("/opt/trn_rl_repo", "/root/.axon_site/_ro/trn_rl_repo"):
    if os.path.isdir(_p) and _p not in sys.path:
        sys.path.insert(0, _p)

from contextlib import ExitStack

import ml_dtypes
import numpy as np

import concourse.bass as bass
import concourse.tile as tile
from concourse import bacc, mybir
from concourse.bass_utils import run_bass_kernel_spmd

BF16 = ml_dtypes.bfloat16
FP8 = ml_dtypes.float8_e4m3

B, S, DM, H, NSYN = 4, 2048, 768, 12, 4
HD = DM // H            # 64
SH = S // 2             # 1024 queries per core
P = 128
NT = S // P             # 16 key tiles
NJ = DM // P            # 6 model-dim tiles
NI2 = DM // 256         # 3 DoubleRow contraction tiles (256 each)
NSH = SH // P           # 8 query tiles
SCALE = float(DM / H) ** -0.5   # 0.125
EPS = 1e-12
N_CORES = 8
WS = 16.0               # host-side weight scale (fp8 range)
VA2W = 784              # padded v_aug row width (12*65 -> %16 for DoubleRow AP)
HEAD_ORDER = list(range(2, H)) + [0, 1]   # heads 0,1 last -> their ctx tile
IT2_ORDER = [1, 2, 0]                     # is contracted last in the out proj

f32 = mybir.dt.float32
bf16 = mybir.dt.bfloat16
fp8 = mybir.dt.float8e4

AF = mybir.ActivationFunctionType
ALU = mybir.AluOpType
DR = mybir.MatmulPerfMode.DoubleRow


def _build_program(bq_nz: bool, bk_nz: bool, bv_nz: bool, mask_nz: bool = False):
    nc = bacc.Bacc(
        "TRN2",
        target_bir_lowering=False,
        debug=False,
        enable_asserts=False,
        num_devices=N_CORES,
    )

    xT = nc.dram_tensor("xT", [P, NI2, 2, S], fp8, kind="ExternalInput").ap()
    xr = nc.dram_tensor("xr", [SH, DM], f32, kind="ExternalInput").ap()
    wq = nc.dram_tensor("wqT", [P, NI2, 2, DM], fp8, kind="ExternalInput").ap()
    wk = nc.dram_tensor("wkT", [P, NI2, 2, DM], fp8, kind="ExternalInput").ap()
    wv = nc.dram_tensor("wvT", [P, NI2, 2, DM], fp8, kind="ExternalInput").ap()
    wo = nc.dram_tensor("woT", [P, NI2, 2, DM], fp8, kind="ExternalInput").ap()
    addikT = nc.dram_tensor("addikT", [NSYN * HD, S], bf16, kind="ExternalInput").ap()
    addiv = nc.dram_tensor("addiv", [S, NSYN * HD], bf16, kind="ExternalInput").ap()
    maskd = nc.dram_tensor("mask", [S], f32, kind="ExternalInput").ap()
    bqd = nc.dram_tensor("bq", [DM], f32, kind="ExternalInput").ap()
    bkd = nc.dram_tensor("bk", [DM], f32, kind="ExternalInput").ap()
    bvd = nc.dram_tensor("bv", [DM], f32, kind="ExternalInput").ap()
    out = nc.dram_tensor("out", [SH, DM], f32, kind="ExternalOutput").ap()

    with tile.TileContext(nc) as tc, ExitStack() as ctx:
        const = ctx.enter_context(tc.tile_pool(name="const", bufs=1))

        xT_sb = const.tile([P, NI2, 2, S], fp8, name="xT_sb")
        wq_sb = const.tile([P, NI2, 2, DM], fp8, name="wq_sb")
        wk_sb = const.tile([P, NI2, 2, DM], fp8, name="wk_sb")
        wv_sb = const.tile([P, NI2, 2, DM], fp8, name="wv_sb")
        wo_sb = const.tile([P, NI2, 2, DM], fp8, name="wo_sb")
        qT_sb = const.tile([P, NJ, SH], bf16, name="qT_sb")
        kT_sb = const.tile([P, NJ, S], bf16, name="kT_sb")
        vaug_sb = const.tile([P, NT // 2, 2, VA2W], fp8, name="vaug_sb")
        ctxT_sb = const.tile([P, NI2, 2, SH], fp8, name="ctxT_sb")
        mask_sb = const.tile([P, NT], f32, name="mask_sb")
        eps_sb = const.tile([P, 1], f32, name="eps_sb")

        # DMA order: q/k weights and x first (they gate the first matmuls).
        for it2 in range(NI2):
            for pl in range(2):
                nc.sync.dma_start(wq_sb[:, it2, pl, :], wq[:, it2, pl, :])
                nc.sync.dma_start(xT_sb[:, it2, pl, :], xT[:, it2, pl, :])
                nc.sync.dma_start(wk_sb[:, it2, pl, :], wk[:, it2, pl, :])
        for it2 in range(NI2):
            for pl in range(2):
                nc.sync.dma_start(wv_sb[:, it2, pl, :], wv[:, it2, pl, :])
        for it2 in range(NI2):
            for pl in range(2):
                nc.sync.dma_start(wo_sb[:, it2, pl, :], wo[:, it2, pl, :])
        nc.sync.dma_start(mask_sb[:], maskd.rearrange("(t p) -> p t", p=P))
        nc.vector.memset(eps_sb[:], EPS)
        # ones columns of v_aug hold 1/16 (exact in fp8); the projection
        # writes below only cover offsets 0..63 of each 65-wide head block.
        nc.gpsimd.memset(vaug_sb[:], 1.0 / WS)

        bias_tiles = {}
        for nz, nm, dram in ((bq_nz, "bq", bqd), (bk_nz, "bk", bkd), (bv_nz, "bv", bvd)):
            if nz:
                t = const.tile([P, NJ], f32, name=f"{nm}_sb")
                nc.sync.dma_start(t[:], dram.rearrange("(t p) -> p t", p=P))
                bias_tiles[nm] = t

        ps = ctx.enter_context(tc.tile_pool(name="ps", bufs=2, space="PSUM"))
        psc = ctx.enter_context(tc.tile_pool(name="psc", bufs=2, space="PSUM"))
        ppool = ctx.enter_context(tc.tile_pool(name="ppool", bufs=4))
        akpool = ctx.enter_context(tc.tile_pool(name="akpool", bufs=2))
        avpool = ctx.enter_context(tc.tile_pool(name="avpool", bufs=2))
        rcpool = ctx.enter_context(tc.tile_pool(name="rcpool", bufs=2))
        bcpool = ctx.enter_context(tc.tile_pool(name="bcpool", bufs=2))
        drpool = ctx.enter_context(tc.tile_pool(name="drpool", bufs=2, space="DRAM"))
        xrpool = ctx.enter_context(tc.tile_pool(name="xrpool", bufs=2))
        hpool = ctx.enter_context(tc.tile_pool(name="hpool", bufs=2))
        opool = ctx.enter_context(tc.tile_pool(name="opool", bufs=2))
        stpool = ctx.enter_context(tc.tile_pool(name="stpool", bufs=3))

        def psum_tile(name):
            return ps.tile([P, 1024], f32, name=name, tag="ps")

        # During the projections the ctx pool is idle, so phase 1 alternates
        # between both psum pools (4 slots) - with fp8 DoubleRow the matmuls
        # are fast enough that 2 slots would leave the PE waiting on psum
        # evacuations (>3.4us gaps -> HAM throttles the PE clock to half).
        _p1n = [0]

        def p1_psum(name):
            _p1n[0] += 1
            if _p1n[0] % 2:
                return ps.tile([P, 1024], f32, name=name, tag="ps")
            return psc.tile([P, 1024], f32, name=name, tag="ctx")

        # ---- Phase 1a: qT[j, s] (16x scaled; own query half = xT cols 0:1024)
        for jt in [1, 2, 3, 4, 5, 0]:
            psq = p1_psum(f"psq{jt}")
            for i, it2 in enumerate(range(NI2)):
                lhs = wq_sb[:, it2, :, jt * P : (jt + 1) * P]
                for c0 in (0, 512):
                    nc.tensor.matmul(
                        psq[:, c0 : c0 + 512],
                        lhsT=lhs,
                        rhs=xT_sb[:, it2, :, c0 : c0 + 512],
                        start=(i == 0),
                        stop=(i == NI2 - 1),
                        perf_mode=DR,
                    )
            dest = qT_sb[:, jt, :]
            if bq_nz:
                nc.scalar.activation(
                    dest, psq[:], AF.Identity, bias=bias_tiles["bq"][:, jt : jt + 1]
                )
            else:
                nc.scalar.copy(out=dest, in_=psq[:])

        # ---- Phase 1b: kT[j, t] (16x; + additive key on heads 0..3) ----
        for jt in [1, 0, 2, 3, 4, 5]:
            for th in range(2):
                psk = p1_psum(f"psk{jt}_{th}")
                for i, it2 in enumerate(range(NI2)):
                    lhs = wk_sb[:, it2, :, jt * P : (jt + 1) * P]
                    for c0 in (0, 512):
                        nc.tensor.matmul(
                            psk[:, c0 : c0 + 512],
                            lhsT=lhs,
                            rhs=xT_sb[:, it2, :, th * 1024 + c0 : th * 1024 + c0 + 512],
                            start=(i == 0),
                            stop=(i == NI2 - 1),
                            perf_mode=DR,
                        )
                dest = kT_sb[:, jt, th * 1024 : (th + 1) * 1024]
                if jt < 2:  # heads 0..3 live on partition tiles 0 and 1
                    ak = akpool.tile([P, 1024], bf16, name="ak", tag="ak")
                    nc.sync.dma_start(
                        ak[:],
                        addikT[jt * P : (jt + 1) * P, th * 1024 : (th + 1) * 1024],
                    )
                    nc.vector.tensor_add(out=dest, in0=psk[:], in1=ak[:])
                    if bk_nz:
                        nc.vector.tensor_scalar_add(
                            dest, dest, bias_tiles["bk"][:, jt : jt + 1]
                        )
                else:
                    if bk_nz:
                        nc.scalar.activation(
                            dest, psk[:], AF.Identity,
                            bias=bias_tiles["bk"][:, jt : jt + 1],
                        )
                    else:
                        nc.scalar.copy(out=dest, in_=psk[:])

        # ---- Phase 1c: v[t, j] (16x) into v_aug (+ additive value) ----
        for tt in range(NT):
            psv = p1_psum(f"psv{tt}")
            for i, it2 in enumerate(range(NI2)):
                lhs = xT_sb[:, it2, :, tt * P : (tt + 1) * P]
                nc.tensor.matmul(
                    psv[:, 0:512], lhsT=lhs, rhs=wv_sb[:, it2, :, 0:512],
                    start=(i == 0), stop=(i == NI2 - 1), perf_mode=DR,
                )
                nc.tensor.matmul(
                    psv[:, 512:768], lhsT=lhs, rhs=wv_sb[:, it2, :, 512:768],
                    start=(i == 0), stop=(i == NI2 - 1), perf_mode=DR,
                )
            vrow = vaug_sb[:, tt // 2, tt % 2, : H * (HD + 1)].rearrange(
                "p (h e) -> p h e", e=HD + 1
            )
            av = avpool.tile([P, NSYN * HD], bf16, name="av", tag="av")
            nc.sync.dma_start(av[:], addiv[tt * P : (tt + 1) * P, :])
            nc.vector.tensor_add(
                out=vrow[:, 0:NSYN, 0:HD],
                in0=psv[:, 0 : NSYN * HD].rearrange("p (h e) -> p h e", e=HD),
                in1=av[:].rearrange("p (h e) -> p h e", e=HD),
            )
            nc.scalar.copy(
                out=vrow[:, NSYN:H, 0:HD],
                in_=psv[:, NSYN * HD : DM].rearrange("p (h e) -> p h e", e=HD),
            )

        # ---- Phase 2: attention per head ----
        ctx_tiles = {}

        def normalize(h):
            it2, pl, po = h // 4, (h % 4) // 2, (h % 2) * HD
            psctx, bc = ctx_tiles.pop(h)
            dest = ctxT_sb[po : po + HD, it2, pl, :]
            nc.vector.tensor_mul(out=dest, in0=psctx[0:HD, :], in1=bc[:])
            if bv_nz:
                nc.vector.tensor_scalar_add(
                    dest, dest, bias_tiles["bv"][po : po + HD, h // 2 : h // 2 + 1]
                )

        for hi, h in enumerate(HEAD_ORDER):
            jt = h // 2
            po = (h % 2) * HD
            kTh = kT_sb[po : po + HD, jt, :]
            qTh = qT_sb[po : po + HD, jt, :]
            psctx = psc.tile([HD + 1, 1024], f32, name=f"ctx{h}", tag="ctx")
            for tp in range(NT // 2):
                pt2 = ppool.tile([P, 2, 1024], fp8, name="pt", tag="pt")
                for lo in (0, 1):
                    tt = 2 * tp + lo
                    pss = psum_tile(f"pss{h}_{tt}")
                    for c0 in (0, 512):
                        nc.tensor.matmul(
                            pss[:, c0 : c0 + 512],
                            lhsT=kTh[:, tt * P : (tt + 1) * P],
                            rhs=qTh[:, c0 : c0 + 512],
                            start=True,
                            stop=True,
                        )
                    if mask_nz:
                        nc.scalar.activation(
                            pt2[:, lo, :], pss[:], AF.Exp,
                            bias=mask_sb[:, tt : tt + 1], scale=SCALE / (WS * WS),
                        )
                    else:
                        nc.scalar.activation(
                            pt2[:, lo, :], pss[:], AF.Exp, scale=SCALE / (WS * WS)
                        )
                for c0 in (0, 512):
                    nc.tensor.matmul(
                        psctx[:, c0 : c0 + 512],
                        lhsT=vaug_sb[:, tp, :, h * (HD + 1) : h * (HD + 1) + HD + 1],
                        rhs=pt2[:, :, c0 : c0 + 512],
                        start=(tp == 0),
                        stop=(tp == NT // 2 - 1),
                        perf_mode=DR,
                    )
            if hi > 0:
                normalize(HEAD_ORDER[hi - 1])
            rc = rcpool.tile([1, 1024], f32, name="rc", tag="rc")
            nc.vector.reciprocal(rc[:], psctx[HD : HD + 1, :])
            dr = drpool.tile([1, 1024], f32, name="dr", tag="dr")
            nc.sync.dma_start(out=dr[:], in_=rc[:])
            bc = bcpool.tile([HD, 1024], f32, name="bc", tag="bc")
            nc.sync.dma_start(out=bc[:], in_=dr.to_broadcast((HD, 1024)))
            ctx_tiles[h] = (psctx, bc)
        normalize(HEAD_ORDER[-1])

        # ---- Phase 3: out proj (psum carries 4096x) + residual + LayerNorm
        for sc in range(NSH):
            pso = psum_tile(f"pso{sc}")
            for i, it2 in enumerate(IT2_ORDER):
                lhs = ctxT_sb[:, it2, :, sc * P : (sc + 1) * P]
                nc.tensor.matmul(
                    pso[:, 0:512], lhsT=lhs, rhs=wo_sb[:, it2, :, 0:512],
                    start=(i == 0), stop=(i == NI2 - 1), perf_mode=DR,
                )
                nc.tensor.matmul(
                    pso[:, 512:768], lhsT=lhs, rhs=wo_sb[:, it2, :, 512:768],
                    start=(i == 0), stop=(i == NI2 - 1), perf_mode=DR,
                )
            xrt = xrpool.tile([P, DM], f32, name="xrt", tag="xr")
            nc.sync.dma_start(xrt[:], xr[sc * P : (sc + 1) * P, :])
            ht = hpool.tile([P, DM], f32, name="ht", tag="h")
            nc.vector.scalar_tensor_tensor(
                out=ht[:], in0=pso[:, 0:DM], scalar=1.0 / 4096.0, in1=xrt[:],
                op0=ALU.mult, op1=ALU.add,
            )
            stats = stpool.tile([P, 3, 6], f32, name="stats", tag="st")
            for g in range(3):
                nc.vector.bn_stats(stats[:, g, :], ht[:, g * 256 : (g + 1) * 256])
            mv = stpool.tile([P, 2], f32, name="mv", tag="mv")
            nc.vector.bn_aggr(mv[:], stats[:])
            sq = stpool.tile([P, 1], f32, name="sq", tag="sq")
            nc.scalar.activation(sq[:], mv[:, 1:2], AF.Sqrt, bias=eps_sb[:])
            rstd = stpool.tile([P, 1], f32, name="rstd", tag="rstd")
            nc.vector.reciprocal(rstd[:], sq[:])
            ot = opool.tile([P, DM], f32, name="ot", tag="ot")
            nc.vector.tensor_scalar(
                out=ot[:], in0=ht[:],
                scalar1=mv[:, 0:1], scalar2=rstd[:],
                op0=ALU.subtract, op1=ALU.mult,
            )
            nc.sync.dma_start(out[sc * P : (sc + 1) * P, :], ot[:])

    nc.compile()
    return nc


_PROGRAM_CACHE: dict = {}


def _get_program(bq_nz, bk_nz, bv_nz, mask_nz=False):
    key = (bq_nz, bk_nz, bv_nz, mask_nz)
    if key not in _PROGRAM_CACHE:
        _PROGRAM_CACHE[key] = _build_program(*key)
    return _PROGRAM_CACHE[key]


def _dr_layout(a):
    """[256*NI2, N] -> [128, NI2, 2, N] DoubleRow K-interleave."""
    n = a.shape[1]
    return np.ascontiguousarray(a.reshape(NI2, 2, P, n).transpose(2, 0, 1, 3))


def _prep_core_inputs(inputs, b, half):
    """Host-side shard prep for core (b, half). Keys are permuted so the core's
    own query half comes first; attention is permutation-invariant in t as long
    as k, v, mask and the additive tensors share the order."""
    x = np.asarray(inputs["hidden_states"][b], np.float32)          # [S, DM]
    if half == 0:
        t_order = slice(None)
        xh = x[:SH]
    else:
        t_order = np.r_[SH:S, 0:SH]
        xh = x[SH:]
    xp = x[t_order] if half else x                                  # [S, DM] permuted
    xT = _dr_layout(np.ascontiguousarray(xp.T)).astype(FP8)
    xr = xh + np.asarray(inputs["bo"], np.float32)[None, :]         # residual + bo
    ak = np.asarray(inputs["addi_key"][b], np.float32) * WS         # [NSYN, S, HD]
    ak = ak.transpose(0, 2, 1).reshape(NSYN * HD, S)
    av = np.asarray(inputs["addi_value"][b], np.float32) * WS
    av = av.transpose(1, 0, 2).reshape(S, NSYN * HD)
    mask = np.asarray(inputs["attention_mask"][b, 0, 0], np.float32)
    if half:
        ak = ak[:, t_order]
        av = av[t_order]
        mask = mask[t_order]
    return {
        "xT": xT,
        "xr": np.ascontiguousarray(xr, np.float32),
        "addikT": np.ascontiguousarray(ak).astype(BF16),
        "addiv": np.ascontiguousarray(av).astype(BF16),
        "mask": np.ascontiguousarray(mask, np.float32),
    }


def _prep_in_maps(inputs):
    def w_prep(w):
        return _dr_layout(
            np.ascontiguousarray(np.asarray(w, np.float32).T) * WS
        ).astype(FP8)

    shared = {
        "wqT": w_prep(inputs["Wq"]),
        "wkT": w_prep(inputs["Wk"]),
        "wvT": w_prep(inputs["Wv"]),
        "woT": w_prep(inputs["Wo"]),
        # biases enter after the 16x-scaled projections / 256x-scaled ctx
        "bq": np.asarray(inputs["bq"], np.float32) * WS,
        "bk": np.asarray(inputs["bk"], np.float32) * WS,
        "bv": np.asarray(inputs["bv"], np.float32) * (WS * WS),
    }
    in_maps = []
    for c in range(N_CORES):
        m = _prep_core_inputs(inputs, c // 2, c % 2)
        m.update(shared)
        in_maps.append(m)
    return in_maps


def _postprocess(inputs, results):
    out = np.empty((B, S, DM), np.float32)
    for c in range(N_CORES):
        b, half = c // 2, c % 2
        out[b, half * SH : (half + 1) * SH] = results[c]["out"]
    ln_g = np.asarray(inputs["ln_g"], np.float32)
    ln_b = np.asarray(inputs["ln_b"], np.float32)
    if np.any(ln_b) or not np.all(ln_g == 1.0):
        out = out * ln_g[None, None, :] + ln_b[None, None, :]
    return out


def run(inputs, trace=False, **kwargs):
    """Run on hardware; returns (full_output, BassKernelResults)."""
    nc = _get_program(
        bool(np.any(inputs["bq"])),
        bool(np.any(inputs["bk"])),
        bool(np.any(inputs["bv"])),
        bool(np.any(inputs["attention_mask"])),
    )
    in_maps = _prep_in_maps(inputs)
    res = run_bass_kernel_spmd(
        nc, in_maps, core_ids=list(range(N_CORES)), trace=trace, **kwargs
    )
    return _postprocess(inputs, res.results), res


def kernel(**inputs) -> np.ndarray:
    out, _ = run(inputs)
    return out
